# revision 4
# baseline (speedup 1.0000x reference)
"""GATNE-T inference kernel for 8 Trainium2 NeuronCores.

Strategy: data-parallel over the batch (1024 samples per core), embedding
tables replicated to every core's HBM. Each core:
  - gathers its neighbor embeddings (per-type, via indirect DMA with a
    constant element offset selecting the edge type),
  - computes the per-type mean, the 4 candidate attention paths (one per
    possible edge type of the sample), selects by the sample's type with
    masks, softmax over types, weighted aggregation, projection,
  - gathers the base embeddings, adds the projection, L2-normalizes.
No collectives are needed; the host only slices the batch and replicates
the (read-only) tables, then concatenates the per-core outputs.
"""
import sys

sys.path.insert(0, "/opt/trn_rl_repo")

import numpy as np

import concourse.bass as bass
import concourse.tile as tile
from concourse import bacc, mybir
from concourse.bass_utils import run_bass_kernel_spmd
from concourse.masks import make_identity

V = 500000
T = 4
D = 32
E = 128
A = 32
B = 8192
S = 10
NCORES = 8
BL = B // NCORES          # 1024 samples per core
P = 128                   # partitions / samples per tile
NTILES = BL // P          # 8 tiles per core
F32 = mybir.dt.float32
I32 = mybir.dt.int32

_cache = {}


def _build():
    nc = bacc.Bacc("TRN2", target_bir_lowering=False, debug=False,
                   num_devices=NCORES)
    nte = nc.dram_tensor("nte", [V, T * D], F32, kind="ExternalInput").ap()
    base = nc.dram_tensor("base", [V, E], F32, kind="ExternalInput").ap()
    tw = nc.dram_tensor("tw", [T * D, E], F32, kind="ExternalInput").ap()
    s1 = nc.dram_tensor("s1", [T, D, A], F32, kind="ExternalInput").ap()
    s2 = nc.dram_tensor("s2", [T, A], F32, kind="ExternalInput").ap()
    tgts = nc.dram_tensor("tgts", [BL, 1], I32, kind="ExternalInput").ap()
    typs = nc.dram_tensor("typs", [BL, 1], I32, kind="ExternalInput").ap()
    nbrs = nc.dram_tensor("nbrs", [BL, T * S], I32, kind="ExternalInput").ap()
    out = nc.dram_tensor("out", [BL, E], F32, kind="ExternalOutput").ap()

    with tile.TileContext(nc) as tc:
        _emit(tc, nc, nte, base, tw, s1, s2, tgts, typs, nbrs, out)
    nc.compile()
    return nc


def _emit(tc, nc, nte, base, tw, s1, s2, tgts, typs, nbrs, out):
    import contextlib

    ctx = contextlib.ExitStack()
    with ctx:
        const = ctx.enter_context(tc.tile_pool(name="const", bufs=1))
        gpool = ctx.enter_context(tc.tile_pool(name="g", bufs=6))
        spool = ctx.enter_context(tc.tile_pool(name="s", bufs=4))
        ppool = ctx.enter_context(tc.tile_pool(name="p", bufs=2, space="PSUM"))

        # ---- setup constants -------------------------------------------
        ident = const.tile([P, P], F32)
        make_identity(nc, ident[:])

        # block-diagonal s1 per weight type w: [ (t,d), (t,a) ]
        s1bd = const.tile([P, T * P], F32)    # 4 blocks of [128,128] side by side
        nc.vector.memset(s1bd[:], 0.0)
        for w in range(T):
            for t in range(T):
                nc.sync.dma_start(
                    out=s1bd[t * D:(t + 1) * D,
                             w * P + t * A: w * P + (t + 1) * A],
                    in_=s1[w])
        # block s2: [ (t,a), (w,t') ] with s2bd[(t,a),(w,t)] = s2[w,a]
        s2bd = const.tile([P, T * T], F32)
        nc.vector.memset(s2bd[:], 0.0)
        for w in range(T):
            for t in range(T):
                nc.sync.dma_start(
                    out=s2bd[t * A:(t + 1) * A, w * T + t: w * T + t + 1],
                    in_=s2[w, :, None])
        # stacked trans_weights [ (w,d), e ]
        twstack = const.tile([P, E], F32)
        nc.sync.dma_start(out=twstack[:], in_=tw[:])

        # ---- load ALL index tiles upfront so the Q7 gather stream never
        # waits on mid-kernel index DMAs ---------------------------------
        nbr_all = const.tile([P, NTILES, T * S], I32)
        nc.sync.dma_start(
            out=nbr_all[:],
            in_=nbrs.rearrange("(i p) j -> p i j", p=P))
        tgt_all = const.tile([P, NTILES], I32)
        nc.sync.dma_start(
            out=tgt_all[:],
            in_=tgts.rearrange("(i p) o -> p (i o)", p=P))
        typ_all = const.tile([P, NTILES], I32)
        nc.sync.dma_start(
            out=typ_all[:],
            in_=typs.rearrange("(i p) o -> p (i o)", p=P))

        for i in range(NTILES):
            r0 = i * P
            nbr_t = nbr_all[:, i, :]
            tgt_t = tgt_all[:, i:i + 1]
            typ_t = typ_all[:, i:i + 1]

            # ---- gather neighbor embeddings ----------------------------
            # g[p, j, :] = nte[nbrs[p, j], t_of(j)*D : (t_of(j)+1)*D]
            g = gpool.tile([P, T * S, D], F32, tag="g")
            for j in range(T * S):
                t_of_j = j // S
                nc.gpsimd.indirect_dma_start(
                    out=g[:, j, :], out_offset=None, in_=nte[:],
                    in_offset=bass.IndirectOffsetOnAxis(
                        ap=nbr_t[:, j:j + 1], axis=0),
                    element_offset=t_of_j * D)
            # ---- gather base embeddings --------------------------------
            base_t = gpool.tile([P, E], F32, tag="base")
            nc.gpsimd.indirect_dma_start(
                out=base_t[:], out_offset=None, in_=base[:],
                in_offset=bass.IndirectOffsetOnAxis(ap=tgt_t[:, 0:1], axis=0))

            # ---- node_agg = mean over S --------------------------------
            agg = spool.tile([P, T * D], F32, tag="agg")
            for t in range(T):
                nc.vector.reduce_sum(
                    agg[:, t * D:(t + 1) * D],
                    g[:, t * S:(t + 1) * S, :].rearrange("p s d -> p d s"),
                    axis=mybir.AxisListType.X)
            nc.scalar.mul(agg[:], agg[:], 1.0 / S)

            # ---- type masks (one-hot of sample type) -------------------
            typf = spool.tile([P, 1], F32, tag="typf")
            nc.vector.tensor_copy(typf[:], typ_t[:])
            masks = spool.tile([P, T], F32, tag="masks")
            for w in range(T):
                nc.vector.tensor_scalar(
                    out=masks[:, w:w + 1], in0=typf[:], scalar1=float(w),
                    scalar2=None, op0=mybir.AluOpType.is_equal)

            # ---- transpose agg -> [(t,d), b] ---------------------------
            aggT_p = ppool.tile([P, P], F32, tag="mm")
            nc.tensor.transpose(out=aggT_p[:], in_=agg[:], identity=ident[:])
            aggT = spool.tile([P, P], F32, tag="aggT_s")
            nc.scalar.copy(aggT[:], aggT_p[:])

            # ---- u_w = tanh(s1bd[w].T @ aggT); scores ------------------
            sc_p = ppool.tile([P, T * T], F32, tag="sc")
            for w in range(T):
                u_p = ppool.tile([P, P], F32, tag="u")
                nc.tensor.matmul(u_p[:], lhsT=s1bd[:, w * P:(w + 1) * P],
                                 rhs=aggT[:], start=True, stop=True)
                u_s = spool.tile([P, P], F32, tag="u_s")
                nc.scalar.activation(u_s[:], u_p[:],
                                     mybir.ActivationFunctionType.Tanh)
                nc.tensor.matmul(sc_p[:, w * T:(w + 1) * T], lhsT=u_s[:],
                                 rhs=s2bd[:, w * T:(w + 1) * T],
                                 start=True, stop=True)

            # ---- select scores by mask, softmax over t -----------------
            scsel = spool.tile([P, T], F32, tag="scsel")
            tmp = spool.tile([P, T], F32, tag="sctmp")
            nc.vector.tensor_tensor(
                out=scsel[:], in0=sc_p[:, 0:T],
                in1=masks[:, 0:1].to_broadcast([P, T]),
                op=mybir.AluOpType.mult)
            for w in range(1, T):
                nc.vector.tensor_tensor(
                    out=tmp[:], in0=sc_p[:, w * T:(w + 1) * T],
                    in1=masks[:, w:w + 1].to_broadcast([P, T]),
                    op=mybir.AluOpType.mult)
                nc.vector.tensor_add(scsel[:], scsel[:], tmp[:])
            ex = spool.tile([P, T], F32, tag="ex")
            nc.scalar.activation(ex[:], scsel[:],
                                 mybir.ActivationFunctionType.Exp)
            sm = spool.tile([P, 1], F32, tag="sm")
            nc.vector.reduce_sum(sm[:], ex[:], axis=mybir.AxisListType.X)
            inv = spool.tile([P, 1], F32, tag="inv")
            nc.vector.reciprocal(inv[:], sm[:])
            att = spool.tile([P, T], F32, tag="att")
            nc.vector.tensor_tensor(out=att[:], in0=ex[:],
                                    in1=inv[:, 0:1].to_broadcast([P, T]),
                                    op=mybir.AluOpType.mult)

            # ---- node_att = sum_t att[b,t] * agg[b,t,:] ----------------
            prod = spool.tile([P, T * D], F32, tag="prod")
            nc.vector.tensor_tensor(
                out=prod[:].rearrange("p (t d) -> p t d", t=T),
                in0=agg[:].rearrange("p (t d) -> p t d", t=T),
                in1=att[:, :, None].to_broadcast([P, T, D]),
                op=mybir.AluOpType.mult)
            natt = spool.tile([P, D], F32, tag="natt")
            nc.vector.reduce_sum(
                natt[:], prod[:].rearrange("p (t d) -> p d t", t=T),
                axis=mybir.AxisListType.X)

            # ---- node_att4[b,(w,d)] = mask_w[b] * natt[b,d] ------------
            natt4 = spool.tile([P, T * D], F32, tag="natt4")
            nc.vector.tensor_tensor(
                out=natt4[:].rearrange("p (w d) -> p w d", w=T),
                in0=natt[:, None, :].to_broadcast([P, T, D]),
                in1=masks[:, :, None].to_broadcast([P, T, D]),
                op=mybir.AluOpType.mult)
            natt4T_p = ppool.tile([P, P], F32, tag="mm")
            nc.tensor.transpose(out=natt4T_p[:], in_=natt4[:],
                                identity=ident[:])
            natt4T = spool.tile([P, P], F32, tag="natt4T_s")
            nc.scalar.copy(natt4T[:], natt4T_p[:])

            # ---- proj = natt4 @ twstack; add base; l2 normalize --------
            proj_p = ppool.tile([P, E], F32, tag="mm")
            nc.tensor.matmul(proj_p[:], lhsT=natt4T[:], rhs=twstack[:],
                             start=True, stop=True)
            sumv = spool.tile([P, E], F32, tag="sumv")
            nc.vector.tensor_add(sumv[:], base_t[:], proj_p[:])
            sq = spool.tile([P, E], F32, tag="sq")
            nc.vector.tensor_tensor(out=sq[:], in0=sumv[:], in1=sumv[:],
                                    op=mybir.AluOpType.mult)
            ssum = spool.tile([P, 1], F32, tag="ssum")
            nc.vector.reduce_sum(ssum[:], sq[:], axis=mybir.AxisListType.X)
            sr = spool.tile([P, 1], F32, tag="sr")
            nc.scalar.activation(sr[:], ssum[:],
                                 mybir.ActivationFunctionType.Sqrt)
            rs = spool.tile([P, 1], F32, tag="rs")
            nc.vector.reciprocal(rs[:], sr[:])
            res = spool.tile([P, E], F32, tag="res")
            nc.vector.tensor_tensor(out=res[:], in0=sumv[:],
                                    in1=rs[:, 0:1].to_broadcast([P, E]),
                                    op=mybir.AluOpType.mult)
            nc.sync.dma_start(out=out[r0:r0 + P, :], in_=res[:])


def get_nc():
    if "nc" not in _cache:
        _cache["nc"] = _build()
    return _cache["nc"]


def kernel(targets, types, neighbors, base_node_embeddings,
           node_type_embeddings, trans_weights, trans_weights_s1,
           trans_weights_s2):
    targets = np.ascontiguousarray(np.asarray(targets, dtype=np.int32))
    types = np.ascontiguousarray(np.asarray(types, dtype=np.int32))
    neighbors = np.ascontiguousarray(np.asarray(neighbors, dtype=np.int32))
    nte = np.ascontiguousarray(
        np.asarray(node_type_embeddings, dtype=np.float32)).reshape(V, T * D)
    base = np.ascontiguousarray(
        np.asarray(base_node_embeddings, dtype=np.float32))
    tw = np.ascontiguousarray(
        np.asarray(trans_weights, dtype=np.float32)).reshape(T * D, E)
    s1 = np.ascontiguousarray(np.asarray(trans_weights_s1, dtype=np.float32))
    s2 = np.ascontiguousarray(
        np.asarray(trans_weights_s2, dtype=np.float32)).reshape(T, A)

    nc = get_nc()
    in_maps = []
    for c in range(NCORES):
        sl = slice(c * BL, (c + 1) * BL)
        in_maps.append({
            "nte": nte,
            "base": base,
            "tw": tw,
            "s1": s1,
            "s2": s2,
            "tgts": targets[sl, None],
            "typs": types[sl, None],
            "nbrs": neighbors[sl].reshape(BL, T * S),
        })
    res = run_bass_kernel_spmd(nc, in_maps, core_ids=list(range(NCORES)))
    return np.concatenate([res.results[c]["out"] for c in range(NCORES)],
                          axis=0)

